# revision 43
# baseline (speedup 1.0000x reference)
"""Trainium2 Bass kernel for ragged KeyQueryAttention pooling.

Math (per batch b):
    logits[t] = sum_l (x @ K)[t,l] * (x @ Q)[t,l]
    att = softmax(logits over valid t)
    out[b]    = sum_t att[t] * x[t, :] + bias        (sum att == 1)

Device strategy (8 NeuronCores, data-parallel over batch):
  - B=64 batches sorted by length (desc), grouped into 8 slots of 8;
    core i takes batch rank 8*j+i for slot j. All cores share one SPMD
    program whose per-slot chunk counts n_j = ceil(max_group_len/128)
    are compiled from the actual lengths (value-specialized; rebuilt per
    call). Rows past each batch's length are masked with -1e30.
  - fp16 on-chip: gpsimd (SWDGE) DMAs cast fp32 HBM -> fp16 SBUF. ALL
    slot loads are issued up front (SBUF holds the full per-core working
    set, ~110KB/partition) so the DMA engines stream flat-out for the
    first ~55us and the back half of the kernel is never starved.
  - Per 128-row chunk: PE transpose (fp16) -> xT in PSUM (4 chunks per
    PSUM bank); ACT copies the 4-pack to SBUF in one op; PE matmul
    xT.T @ A -> H in PSUM (fp32, A = (K Q^T + Q K^T)/2 host-built);
    one DVE scalar_tensor_tensor (H * x, row-sum accum) -> logits
    column.
  - Per slot: additive mask + row max (DVE), global max via PE
    transpose + reduce, ACT exp with accum_out giving per-partition Z,
    then PE weighted-sum matmuls (x chunk stationary, att column
    moving) accumulating acc[128,1] in PSUM.
  - Slots are software-pipelined (chunk phase j, softmax tail j-1,
    weighted sum j-1). Slot order: ascending sizes for a cheap
    prologue, the big slots next (descending), and the smallest slot
    last so the exposed final softmax/weighted-sum tail is tiny.
  - Host: out = acc / sum(zrow) + bias, un-permute batches.

Measured dead ends (do not revisit): DMA-xbar SBUF->SBUF transposes
(blocks the HWDGE sequencer ~1.1us/op; batched per half-slot it still
stretches the load stream -> 158us vs 118us), ACT Square-accum rowdot
(~830ns/chunk vs DVE 270ns), kq=[K|Q] split dot (DVE may read only one
PSUM operand), fp16 H (TRN2 matmul output must be fp32), psT=2/psG=5
and quarter-split loads (132us vs 118us).
"""

import os
import numpy as np

import concourse.bass as bass
import concourse.bacc as bacc
import concourse.tile as tile
from concourse import mybir
from concourse import bass_isa
from concourse.bass_utils import run_bass_kernel_spmd
from concourse.masks import make_identity

B, T, D, L = 64, 8192, 128, 64
NCORES = 8
SLOTS = B // NCORES  # 8 slots per core
F32 = mybir.dt.float32
F16 = mybir.dt.float16

LAST_EXEC_NS = None  # filled when KQA_TRACE=1

# Slot positions (processing order) whose transposes go through the DMA
# xbar instead of the PE: the late big slots run after the HBM loads have
# drained, when the DMA engines are otherwise idle.
XBAR_POS = frozenset()  # xbar DMA transposes measured slower; keep PE
# Fraction of each slot's chunks whose rowdot runs on ACT (two Square
# accums) instead of DVE.  Measured ~830ns/chunk on ACT vs ~270ns on
# DVE — keep everything on the DVE.
ACT_DOT_FRAC = {}

_PROG_CACHE = {}


def _build_program(n_list):
    nc = bacc.Bacc()
    ntot = sum(n_list)
    xs = [
        nc.declare_dram_parameter(f"x{j}", [n, 128, D], F32, isOutput=False)
        for j, n in enumerate(n_list)
    ]
    amat = nc.declare_dram_parameter("amat", [D, D], F32, isOutput=False)
    kqpm = nc.declare_dram_parameter("kqpm", [D, 2 * L], F32, isOutput=False)
    maskp = nc.declare_dram_parameter("mask", [128, ntot], F32, isOutput=False)
    outp = nc.declare_dram_parameter("out", [128, 2 * SLOTS], F32, isOutput=True)

    AF = mybir.ActivationFunctionType
    ALU = mybir.AluOpType

    with tile.TileContext(nc) as tc:
        with (
            tc.tile_pool(name="consts", bufs=1) as consts,
            tc.tile_pool(name="xpool", bufs=1) as xpool,
            tc.tile_pool(name="tpool", bufs=34) as tpool,
            tc.tile_pool(name="jpool", bufs=2) as jpool,
            tc.tile_pool(name="spool", bufs=3) as spool,
            tc.tile_pool(name="psT", bufs=3, space="PSUM") as psT,
            tc.tile_pool(name="psG", bufs=4, space="PSUM") as psG,
            tc.tile_pool(name="psX", bufs=1, space="PSUM") as psX,
        ):
            id16 = consts.tile([128, 128], F16)
            make_identity(nc, id16)
            id32 = consts.tile([128, 128], F32)
            make_identity(nc, id32)
            neg_row = consts.tile([1, 128], F32)
            nc.vector.memset(neg_row, -1.0)
            a_f32 = consts.tile([D, D], F32)
            nc.sync.dma_start(out=a_f32, in_=amat[:, :])
            a_sb = consts.tile([D, D], F16)
            nc.vector.tensor_copy(a_sb, a_f32)
            kqpm_f32 = consts.tile([D, 2 * L], F32)
            nc.sync.dma_start(out=kqpm_f32, in_=kqpm[:, :])
            kqpm_sb = consts.tile([D, 2 * L], F16)
            nc.vector.tensor_copy(kqpm_sb, kqpm_f32)
            mask_sb = consts.tile([128, ntot], F32)
            nc.sync.dma_start(out=mask_sb, in_=maskp[:, :])
            out_sb = consts.tile([128, 2 * SLOTS], F32)
            # one PSUM bank shared by the max-chain and the wsum accumulator
            # (disjoint columns; serial per-slot use)
            amx_tile = psX.tile([128, 512], F32, tag="amx")

            off = [0] * SLOTS
            o = 0
            for j, n in enumerate(n_list):
                off[j] = o
                o += n

            x_sbs = [None] * SLOTS
            p_sbs = [None] * SLOTS
            logits_sbs = [None] * SLOTS
            act_parts = [None] * SLOTS
            xbar_tiles = [None] * SLOTS  # [(c0, c1, tile)] per xbar slot

            def load(j):
                n = n_list[j]
                x_sb = xpool.tile([128, n, D], F16, tag=f"x{j}", bufs=1)
                x_sbs[j] = x_sb
                # SWDGE (gpsimd) DMA casts fp32 DRAM -> fp16 SBUF on the fly.
                if n >= 6:
                    h = (n + 1) // 2
                    pieces = ((0, h), (h, n))
                else:
                    pieces = ((0, n),)
                use_xbar = j in XBAR_POS
                if use_xbar:
                    xbar_tiles[j] = []
                for c0, c1 in pieces:
                    nc.gpsimd.dma_start(
                        out=x_sb[:, c0:c1, :],
                        in_=xs[j][c0:c1, :, :].rearrange("c t d -> t c d"),
                    )
                    if use_xbar:
                        # Batched per-chunk transpose on the DMA xbar right
                        # behind the half-load: out[d, c, t] = x[t, c, d].
                        # Runs on the otherwise-idle Sync HWDGE queue.
                        w = c1 - c0
                        xT = tpool.tile(
                            [128, w, 128], F16, tag=f"xb{j}_{c0}", bufs=1
                        )
                        nc.sync.dma_start(
                            out=xT,
                            in_=x_sb[:, c0:c1, :],
                            transpose=True,
                        )
                        xbar_tiles[j].append((c0, c1, xT))

            PW = 4  # chunks per transpose/copy group (one PSUM bank)

            def t_group_ops(j):
                """Transpose emitters for slot j; returns (ops, getxT, groups).

                XBAR_POS slots already transposed on the DMA xbar at load
                time (no PE/ACT work here); others transpose on PE into
                PSUM and ACT-copy each PW-chunk pack to SBUF.
                """
                n = n_list[j]
                x_sb = x_sbs[j]
                groups = [(c, min(c + PW, n) - c) for c in range(0, n, PW)]

                if j in XBAR_POS:
                    pieces = xbar_tiles[j]

                    def getxT(c):
                        for c0, c1, xT in pieces:
                            if c0 <= c < c1:
                                return xT[:, c - c0, :]
                        raise AssertionError(c)

                    return [], getxT, groups

                tiles = [None] * len(groups)

                def mk(k):
                    def f():
                        c0, w = groups[k]
                        xT_sb = tpool.tile([128, PW, 128], F16, tag="xTs")
                        tiles[k] = xT_sb
                        xT_ps = psT.tile([128, PW, 128], F16, tag="xT")
                        for i in range(w):
                            nc.tensor.transpose(
                                xT_ps[:, i, :], x_sb[:, c0 + i, :], id16
                            )
                        nc.scalar.copy(xT_sb[:, :w, :], xT_ps[:, :w, :])

                    return f

                def getxT(c):
                    return tiles[c // PW][:, c % PW, :]

                return [mk(k) for k in range(len(groups))], getxT, groups

            def h_group_ops(j, getxT, groups):
                """Projection matmul + rowdot emitters, one per group.

                Chunks below n_dve use H = x@A on PE and a DVE
                H*x row-sum.  The rest use sr = x@[(K+Q)/2|(K-Q)/2] and two
                ACT Square ops with accum (logits = sum s^2 - sum r^2),
                merged per slot; this offloads the rowdot from the DVE to
                the otherwise-idle ACT in the xbar phase.
                """
                x_sb = x_sbs[j]
                n = n_list[j]
                n_dve = n - int(round(n * ACT_DOT_FRAC.get(j, 0.0)))
                logits = spool.tile([128, n], F32, tag="logits")
                logits_sbs[j] = logits
                n_act = n - n_dve
                if n_act:
                    lp = spool.tile([128, n_act], F32, tag="lp")
                    ln_ = spool.tile([128, n_act], F32, tag="ln")
                    act_parts[j] = (lp, ln_, n_dve)
                else:
                    act_parts[j] = None

                def mk(k):
                    def f():
                        c0, w = groups[k]
                        for i in range(w):
                            c = c0 + i
                            g_ps = psG.tile([128, D], F32, tag="g")
                            if c < n_dve:
                                nc.tensor.matmul(
                                    g_ps, getxT(c), a_sb,
                                    start=True, stop=True,
                                )
                                junk = jpool.tile([128, D], F16, tag="junk")
                                # logits[t, c] = sum_d H[t, d] * x[t, d]
                                nc.vector.scalar_tensor_tensor(
                                    out=junk,
                                    in0=g_ps,
                                    scalar=1.0,
                                    in1=x_sb[:, c, :],
                                    op0=ALU.mult,
                                    op1=ALU.mult,
                                    accum_out=logits[:, c : c + 1],
                                )
                            else:
                                nc.tensor.matmul(
                                    g_ps, getxT(c), kqpm_sb,
                                    start=True, stop=True,
                                )
                                ca = c - n_dve
                                junkA = jpool.tile([128, L], F16, tag="junkA")
                                nc.scalar.activation(
                                    junkA,
                                    g_ps[:, 0:L],
                                    AF.Square,
                                    accum_out=lp[:, ca : ca + 1],
                                )
                                junkB = jpool.tile([128, L], F16, tag="junkB")
                                nc.scalar.activation(
                                    junkB,
                                    g_ps[:, L : 2 * L],
                                    AF.Square,
                                    accum_out=ln_[:, ca : ca + 1],
                                )

                    return f

                return [mk(k) for k in range(len(groups))]

            def mask_rowmax(j):
                logits = logits_sbs[j]
                n = n_list[j]
                if act_parts[j] is not None:
                    lp, ln_, n_dve = act_parts[j]
                    nc.vector.tensor_tensor(
                        logits[:, n_dve:n], lp, ln_, op=ALU.subtract
                    )
                nc.vector.tensor_tensor(
                    logits, logits, mask_sb[:, off[j] : off[j] + n], op=ALU.add
                )
                rowmax = spool.tile([128, 1], F32, tag="rmax")
                nc.vector.tensor_reduce(
                    rowmax, logits, axis=mybir.AxisListType.X, op=ALU.max
                )
                return rowmax

            def tail_a(j):
                # global max row: PE transpose of rowmax into the shared bank
                nc.tensor.transpose(amx_tile[0:1, 0:128], rowmaxes[j], id32)

            def tail_b(j):
                maxs = spool.tile([1, 1], F32, tag="maxs")
                nc.vector.tensor_reduce(
                    maxs,
                    amx_tile[0:1, 0:128],
                    axis=mybir.AxisListType.X,
                    op=ALU.max,
                )
                nc.tensor.matmul(
                    amx_tile[:, 200:201], neg_row, maxs, start=True, stop=True
                )
                negm = spool.tile([128, 1], F32, tag="negm")
                nc.vector.tensor_copy(negm, amx_tile[:, 200:201])
                p_sb = spool.tile([128, n_list[j]], F16, tag="p")
                p_sbs[j] = p_sb
                nc.scalar.activation(
                    p_sb,
                    logits_sbs[j],
                    AF.Exp,
                    bias=negm,
                    scale=1.0,
                    accum_out=out_sb[:, SLOTS + j : SLOTS + j + 1],
                )

            def wsum(j):
                n = n_list[j]
                acc_ps = amx_tile[:, 300:301]
                for c in range(n):
                    nc.tensor.matmul(
                        acc_ps,
                        x_sbs[j][:, c, :],
                        p_sbs[j][:, c : c + 1],
                        start=(c == 0),
                        stop=(c == n - 1),
                    )
                nc.scalar.copy(out_sb[:, j : j + 1], acc_ps)

            # Load everything up front: the DMA engines stream continuously
            # and compute is never starved mid-kernel.
            for j in range(SLOTS):
                load(j)

            # Cross-slot pipeline: phase j runs kq/rowdot of slot j
            # interleaved with transposes/copies of slot j+1 (whose results
            # are consumed one phase later), plus the softmax tail of j-1
            # and the weighted sum of j-1.
            rowmaxes = [None] * SLOTS
            tinfo = [None] * SLOTS
            tops, getxT, groups = t_group_ops(0)
            tinfo[0] = (getxT, groups)
            for op in tops:
                op()
            for j in range(SLOTS):
                if j + 1 < SLOTS:
                    tops, getxT, groups = t_group_ops(j + 1)
                    tinfo[j + 1] = (getxT, groups)
                else:
                    tops = []
                hops = h_group_ops(j, *tinfo[j])
                m = max(len(tops), len(hops))
                stage = 0
                for gs in range(m):
                    # H before next-slot transposes: a transpose whose input
                    # chunk hasn't landed from HBM yet would head-of-line
                    # block ready H matmuls in the in-order PE queue.
                    if gs < len(hops):
                        hops[gs]()
                    if j >= 1:
                        if stage == 0 and gs >= max(1, (2 * m) // 5):
                            tail_a(j - 1)
                            stage = 1
                        elif stage == 1 and gs >= max(2, (11 * m) // 20):
                            tail_b(j - 1)
                            stage = 2
                        elif stage == 2 and gs >= min(max(4, (3 * m) // 4), m - 1):
                            wsum(j - 1)
                            stage = 3
                    if gs < len(tops):
                        tops[gs]()
                if j >= 1:
                    if stage == 0:
                        tail_a(j - 1)
                        stage = 1
                    if stage == 1:
                        tail_b(j - 1)
                        stage = 2
                    if stage == 2:
                        wsum(j - 1)
                rowmaxes[j] = mask_rowmax(j)
            tail_a(SLOTS - 1)
            tail_b(SLOTS - 1)
            wsum(SLOTS - 1)
            nc.sync.dma_start(out=outp[:, :], in_=out_sb)
    nc.finalize()
    return nc


def kernel(seq, lengths, key_w, query_w, bias):
    global LAST_EXEC_NS
    seq = np.asarray(seq, dtype=np.float32)
    lengths_np = np.asarray(lengths).astype(np.int64)
    key_w = np.asarray(key_w, dtype=np.float32)
    query_w = np.asarray(query_w, dtype=np.float32)
    bias = np.asarray(bias, dtype=np.float32)

    order = np.argsort(-lengths_np, kind="stable")  # descending length
    n_desc = []
    for j in range(SLOTS):
        grp = order[j * NCORES : (j + 1) * NCORES]
        n_desc.append(max(1, int(-(-int(lengths_np[grp].max()) // 128))))
    # Small slots first (cheap prologue, PE transposes while the loads
    # stream), then the big slots descending (xbar transposes on the
    # now-idle DMA engines), with the smallest slot last so the exposed
    # final softmax/weighted-sum tail is tiny.
    asc = sorted(range(SLOTS), key=lambda j: n_desc[j])
    slot_perm = (
        asc[1 : SLOTS - 3]
        + [asc[SLOTS - 1], asc[SLOTS - 2], asc[SLOTS - 3], asc[0]]
    )
    n_list = [n_desc[j] for j in slot_perm]
    key = tuple(n_list)
    if key not in _PROG_CACHE:
        _PROG_CACHE[key] = _build_program(n_list)
    nc = _PROG_CACHE[key]

    amat_np = (key_w @ query_w.T + query_w @ key_w.T) * 0.5  # [D, D] symmetric
    # logits = |x(K+Q)/2|^2 - |x(K-Q)/2|^2 exactly equals (xK).(xQ)
    kqpm_np = np.ascontiguousarray(
        np.concatenate([(key_w + query_w) * 0.5, (key_w - query_w) * 0.5], axis=1)
    )
    in_maps = []
    for i in range(NCORES):
        m = {"amat": amat_np, "kqpm": kqpm_np}
        mask_cols = []
        for js, jd in enumerate(slot_perm):
            n = n_list[js]
            b = int(order[jd * NCORES + i])
            m[f"x{js}"] = seq[b, : n * 128, :].reshape(n, 128, D)
            lb = int(lengths_np[b])
            col = np.where(np.arange(n * 128) < lb, 0.0, -1e30).astype(np.float32)
            mask_cols.append(col.reshape(n, 128).T)  # [128, n]
        m["mask"] = np.ascontiguousarray(np.concatenate(mask_cols, axis=1))
        in_maps.append(m)

    trace = os.environ.get("KQA_TRACE") == "1"
    res = run_bass_kernel_spmd(
        nc, in_maps, core_ids=list(range(NCORES)), trace=trace
    )
    LAST_EXEC_NS = res.exec_time_ns

    out = np.empty((B, D), dtype=np.float32)
    for i in range(NCORES):
        r = res.results[i]["out"]  # [128, 2*SLOTS]
        for js, jd in enumerate(slot_perm):
            b = int(order[jd * NCORES + i])
            acc = r[:, js]
            z = r[:, SLOTS + js].sum(dtype=np.float64)
            out[b] = (acc / z).astype(np.float32) + bias
    return out


# revision 49
# speedup vs baseline: 1.0077x; 1.0077x over previous
"""Trainium2 Bass kernel for ragged KeyQueryAttention pooling.

Math (per batch b):
    logits[t] = sum_l (x @ K)[t,l] * (x @ Q)[t,l]
    att = softmax(logits over valid t)
    out[b]    = sum_t att[t] * x[t, :] + bias        (sum att == 1)

Device strategy (8 NeuronCores, data-parallel over batch):
  - B=64 batches sorted by length (desc), grouped into 8 slots of 8;
    core i takes batch rank 8*j+i for slot j. All cores share one SPMD
    program whose per-slot chunk counts n_j = ceil(max_group_len/128)
    are compiled from the actual lengths (value-specialized; rebuilt per
    call). Rows past each batch's length are masked with -1e30.
  - fp16 on-chip: gpsimd (SWDGE) DMAs cast fp32 HBM -> fp16 SBUF. ALL
    slot loads are issued up front (SBUF holds the full per-core working
    set, ~110KB/partition) so the DMA engines stream flat-out for the
    first ~55us and the back half of the kernel is never starved.
  - Per 128-row chunk: PE transpose (fp16) -> xT in PSUM (4 chunks per
    PSUM bank); ACT copies the 4-pack to SBUF in one op; PE matmul
    xT.T @ A -> H in PSUM (fp32, A = (K Q^T + Q K^T)/2 host-built);
    one DVE scalar_tensor_tensor (H * x, row-sum accum) -> logits
    column.
  - Per slot: additive mask + row max (DVE), global max via PE
    transpose + reduce, ACT exp with accum_out giving per-partition Z,
    then PE weighted-sum matmuls (x chunk stationary, att column
    moving) accumulating acc[128,1] in PSUM.
  - Slots are software-pipelined (chunk phase j, softmax tail j-1,
    weighted sum j-1). Slot order: ascending sizes for a cheap
    prologue, the big slots next (descending), and the smallest slot
    last so the exposed final softmax/weighted-sum tail is tiny.
  - Host: out = acc / sum(zrow) + bias, un-permute batches.

Measured dead ends (do not revisit): DMA-xbar SBUF->SBUF transposes
(blocks the HWDGE sequencer ~1.1us/op; batched per half-slot it still
stretches the load stream -> 158us vs 118us), ACT Square-accum rowdot
(~830ns/chunk vs DVE 270ns), kq=[K|Q] split dot (DVE may read only one
PSUM operand), fp16 H (TRN2 matmul output must be fp32), psT=2/psG=5
and quarter-split loads (132us vs 118us).
"""

import os
import numpy as np

import concourse.bass as bass
import concourse.bacc as bacc
import concourse.tile as tile
from concourse import mybir
from concourse import bass_isa
from concourse.bass_utils import run_bass_kernel_spmd
from concourse.masks import make_identity

B, T, D, L = 64, 8192, 128, 64
NCORES = 8
SLOTS = B // NCORES  # 8 slots per core
F32 = mybir.dt.float32
F16 = mybir.dt.float16

LAST_EXEC_NS = None  # filled when KQA_TRACE=1

# Slot positions (processing order) whose transposes go through the DMA
# xbar instead of the PE: the late big slots run after the HBM loads have
# drained, when the DMA engines are otherwise idle.
XBAR_POS = frozenset()  # xbar DMA transposes measured slower; keep PE
# Every k-th chunk of these slots computes its rowdot on ACT (two
# Square accums over sr = x@[(K+Q)/2|(K-Q)/2]) instead of DVE.
# Measured WORSE in every arrangement (suffix +6us, interleaved 1-in-5
# +10us): the Square ops sit in the in-order ACT queue between
# transpose-pack copies and head-of-line block them, stalling psT and
# then the PE.  Keep empty.
ACT_DOT_EVERY = {}

_PROG_CACHE = {}


def _build_program(n_list):
    nc = bacc.Bacc()
    ntot = sum(n_list)
    xs = [
        nc.declare_dram_parameter(f"x{j}", [n, 128, D], F32, isOutput=False)
        for j, n in enumerate(n_list)
    ]
    amat = nc.declare_dram_parameter("amat", [D, D], F32, isOutput=False)
    kqpm = nc.declare_dram_parameter("kqpm", [D, 2 * L], F32, isOutput=False)
    maskp = nc.declare_dram_parameter("mask", [128, ntot], F32, isOutput=False)
    outp = nc.declare_dram_parameter("out", [128, 2 * SLOTS], F32, isOutput=True)

    AF = mybir.ActivationFunctionType
    ALU = mybir.AluOpType

    with tile.TileContext(nc) as tc:
        with (
            tc.tile_pool(name="consts", bufs=1) as consts,
            tc.tile_pool(name="xpool", bufs=1) as xpool,
            tc.tile_pool(name="tpool", bufs=34) as tpool,
            tc.tile_pool(name="jpool", bufs=2) as jpool,
            tc.tile_pool(name="spool", bufs=3) as spool,
            tc.tile_pool(name="psT", bufs=3, space="PSUM") as psT,
            tc.tile_pool(name="psG", bufs=4, space="PSUM") as psG,
            tc.tile_pool(name="psX", bufs=1, space="PSUM") as psX,
        ):
            id16 = consts.tile([128, 128], F16)
            make_identity(nc, id16)
            id32 = consts.tile([128, 128], F32)
            make_identity(nc, id32)
            neg_row = consts.tile([1, 128], F32)
            nc.vector.memset(neg_row, -1.0)
            a_f32 = consts.tile([D, D], F32)
            nc.sync.dma_start(out=a_f32, in_=amat[:, :])
            a_sb = consts.tile([D, D], F16)
            nc.vector.tensor_copy(a_sb, a_f32)
            kqpm_f32 = consts.tile([D, 2 * L], F32)
            nc.sync.dma_start(out=kqpm_f32, in_=kqpm[:, :])
            kqpm_sb = consts.tile([D, 2 * L], F16)
            nc.vector.tensor_copy(kqpm_sb, kqpm_f32)
            mask_sb = consts.tile([128, ntot], F32)
            nc.sync.dma_start(out=mask_sb, in_=maskp[:, :])
            out_sb = consts.tile([128, 2 * SLOTS], F32)
            # one PSUM bank shared by the max-chain and the wsum accumulator
            # (disjoint columns; serial per-slot use)
            amx_tile = psX.tile([128, 512], F32, tag="amx")

            off = [0] * SLOTS
            o = 0
            for j, n in enumerate(n_list):
                off[j] = o
                o += n

            x_sbs = [None] * SLOTS
            p_sbs = [None] * SLOTS
            logits_sbs = [None] * SLOTS
            act_parts = [None] * SLOTS
            xbar_tiles = [None] * SLOTS  # [(c0, c1, tile)] per xbar slot

            def load(j):
                n = n_list[j]
                x_sb = xpool.tile([128, n, D], F16, tag=f"x{j}", bufs=1)
                x_sbs[j] = x_sb
                # SWDGE (gpsimd) DMA casts fp32 DRAM -> fp16 SBUF on the fly.
                if n >= 6:
                    h = (n + 1) // 2
                    pieces = ((0, h), (h, n))
                else:
                    pieces = ((0, n),)
                use_xbar = j in XBAR_POS
                if use_xbar:
                    xbar_tiles[j] = []
                for c0, c1 in pieces:
                    nc.gpsimd.dma_start(
                        out=x_sb[:, c0:c1, :],
                        in_=xs[j][c0:c1, :, :].rearrange("c t d -> t c d"),
                    )
                    if use_xbar:
                        # Batched per-chunk transpose on the DMA xbar right
                        # behind the half-load: out[d, c, t] = x[t, c, d].
                        # Runs on the otherwise-idle Sync HWDGE queue.
                        w = c1 - c0
                        xT = tpool.tile(
                            [128, w, 128], F16, tag=f"xb{j}_{c0}", bufs=1
                        )
                        nc.sync.dma_start(
                            out=xT,
                            in_=x_sb[:, c0:c1, :],
                            transpose=True,
                        )
                        xbar_tiles[j].append((c0, c1, xT))

            PW = 4  # chunks per transpose/copy group (one PSUM bank)

            def t_group_ops(j):
                """Transpose emitters for slot j; returns (ops, getxT, groups).

                XBAR_POS slots already transposed on the DMA xbar at load
                time (no PE/ACT work here); others transpose on PE into
                PSUM and ACT-copy each PW-chunk pack to SBUF.
                """
                n = n_list[j]
                x_sb = x_sbs[j]
                groups = [(c, min(c + PW, n) - c) for c in range(0, n, PW)]

                if j in XBAR_POS:
                    pieces = xbar_tiles[j]

                    def getxT(c):
                        for c0, c1, xT in pieces:
                            if c0 <= c < c1:
                                return xT[:, c - c0, :]
                        raise AssertionError(c)

                    return [], getxT, groups

                tiles = [None] * len(groups)

                def mk(k):
                    def f():
                        c0, w = groups[k]
                        xT_sb = tpool.tile([128, PW, 128], F16, tag="xTs")
                        tiles[k] = xT_sb
                        xT_ps = psT.tile([128, PW, 128], F16, tag="xT")
                        for i in range(w):
                            nc.tensor.transpose(
                                xT_ps[:, i, :], x_sb[:, c0 + i, :], id16
                            )
                        nc.scalar.copy(xT_sb[:, :w, :], xT_ps[:, :w, :])

                    return f

                def getxT(c):
                    return tiles[c // PW][:, c % PW, :]

                return [mk(k) for k in range(len(groups))], getxT, groups

            def h_group_ops(j, getxT, groups):
                """Projection matmul + rowdot emitters, one per group.

                Chunks below n_dve use H = x@A on PE and a DVE
                H*x row-sum.  The rest use sr = x@[(K+Q)/2|(K-Q)/2] and two
                ACT Square ops with accum (logits = sum s^2 - sum r^2),
                merged per slot; this offloads the rowdot from the DVE to
                the otherwise-idle ACT in the xbar phase.
                """
                x_sb = x_sbs[j]
                n = n_list[j]
                every = ACT_DOT_EVERY.get(j, 0)
                logits = spool.tile([128, n], F32, tag="logits")
                logits_sbs[j] = logits
                if every:
                    # Full-width, zeroed; ACT accums land at their true
                    # columns and the per-slot merge adds (lp - ln) into
                    # logits (zero on DVE columns).  logits is zeroed too:
                    # its ACT columns are only written by the merge.
                    nc.vector.memset(logits, 0.0)
                    lp = spool.tile([128, n], F32, tag="lp")
                    ln_ = spool.tile([128, n], F32, tag="ln")
                    nc.vector.memset(lp, 0.0)
                    nc.vector.memset(ln_, 0.0)
                    act_parts[j] = (lp, ln_)
                else:
                    act_parts[j] = None

                def mk(k):
                    def f():
                        c0, w = groups[k]
                        for i in range(w):
                            c = c0 + i
                            g_ps = psG.tile([128, D], F32, tag="g")
                            if not every or (c % every) != every - 1:
                                nc.tensor.matmul(
                                    g_ps, getxT(c), a_sb,
                                    start=True, stop=True,
                                )
                                junk = jpool.tile([128, D], F16, tag="junk")
                                # logits[t, c] = sum_d H[t, d] * x[t, d]
                                nc.vector.scalar_tensor_tensor(
                                    out=junk,
                                    in0=g_ps,
                                    scalar=1.0,
                                    in1=x_sb[:, c, :],
                                    op0=ALU.mult,
                                    op1=ALU.mult,
                                    accum_out=logits[:, c : c + 1],
                                )
                            else:
                                nc.tensor.matmul(
                                    g_ps, getxT(c), kqpm_sb,
                                    start=True, stop=True,
                                )
                                junkA = jpool.tile([128, L], F16, tag="junkA")
                                nc.scalar.activation(
                                    junkA,
                                    g_ps[:, 0:L],
                                    AF.Square,
                                    accum_out=lp[:, c : c + 1],
                                )
                                junkB = jpool.tile([128, L], F16, tag="junkB")
                                nc.scalar.activation(
                                    junkB,
                                    g_ps[:, L : 2 * L],
                                    AF.Square,
                                    accum_out=ln_[:, c : c + 1],
                                )

                    return f

                return [mk(k) for k in range(len(groups))]

            def mask_rowmax(j):
                logits = logits_sbs[j]
                n = n_list[j]
                if act_parts[j] is not None:
                    lp, ln_ = act_parts[j]
                    nc.vector.tensor_tensor(lp, lp, ln_, op=ALU.subtract)
                    nc.vector.tensor_tensor(logits, logits, lp, op=ALU.add)
                nc.vector.tensor_tensor(
                    logits, logits, mask_sb[:, off[j] : off[j] + n], op=ALU.add
                )
                rowmax = spool.tile([128, 1], F32, tag="rmax")
                nc.vector.tensor_reduce(
                    rowmax, logits, axis=mybir.AxisListType.X, op=ALU.max
                )
                return rowmax

            def tail_a(j):
                # global max row: PE transpose of rowmax into the shared bank
                nc.tensor.transpose(amx_tile[0:1, 0:128], rowmaxes[j], id32)

            def tail_b(j):
                maxs = spool.tile([1, 1], F32, tag="maxs")
                nc.vector.tensor_reduce(
                    maxs,
                    amx_tile[0:1, 0:128],
                    axis=mybir.AxisListType.X,
                    op=ALU.max,
                )
                nc.tensor.matmul(
                    amx_tile[:, 200:201], neg_row, maxs, start=True, stop=True
                )
                negm = spool.tile([128, 1], F32, tag="negm")
                nc.vector.tensor_copy(negm, amx_tile[:, 200:201])
                p_sb = spool.tile([128, n_list[j]], F16, tag="p")
                p_sbs[j] = p_sb
                nc.scalar.activation(
                    p_sb,
                    logits_sbs[j],
                    AF.Exp,
                    bias=negm,
                    scale=1.0,
                    accum_out=out_sb[:, SLOTS + j : SLOTS + j + 1],
                )

            def wsum(j):
                n = n_list[j]
                acc_ps = amx_tile[:, 300:301]
                for c in range(n):
                    nc.tensor.matmul(
                        acc_ps,
                        x_sbs[j][:, c, :],
                        p_sbs[j][:, c : c + 1],
                        start=(c == 0),
                        stop=(c == n - 1),
                    )
                nc.scalar.copy(out_sb[:, j : j + 1], acc_ps)

            # Load everything up front: the DMA engines stream continuously
            # and compute is never starved mid-kernel.
            for j in range(SLOTS):
                load(j)

            # Cross-slot pipeline: phase j runs kq/rowdot of slot j
            # interleaved with transposes/copies of slot j+1 (whose results
            # are consumed one phase later), plus the softmax tail of j-1
            # and the weighted sum of j-1.
            rowmaxes = [None] * SLOTS
            tinfo = [None] * SLOTS
            tops, getxT, groups = t_group_ops(0)
            tinfo[0] = (getxT, groups)
            for op in tops:
                op()
            for j in range(SLOTS):
                if j + 1 < SLOTS:
                    tops, getxT, groups = t_group_ops(j + 1)
                    tinfo[j + 1] = (getxT, groups)
                else:
                    tops = []
                hops = h_group_ops(j, *tinfo[j])
                m = max(len(tops), len(hops))
                stage = 0
                for gs in range(m):
                    # H before next-slot transposes: a transpose whose input
                    # chunk hasn't landed from HBM yet would head-of-line
                    # block ready H matmuls in the in-order PE queue.
                    if gs < len(hops):
                        hops[gs]()
                    if j >= 1:
                        if stage == 0 and gs >= max(1, (2 * m) // 5):
                            tail_a(j - 1)
                            stage = 1
                        elif stage == 1 and gs >= max(2, (11 * m) // 20):
                            tail_b(j - 1)
                            stage = 2
                        elif stage == 2 and gs >= min(max(4, (3 * m) // 4), m - 1):
                            wsum(j - 1)
                            stage = 3
                    if gs < len(tops):
                        tops[gs]()
                if j >= 1:
                    if stage == 0:
                        tail_a(j - 1)
                        stage = 1
                    if stage == 1:
                        tail_b(j - 1)
                        stage = 2
                    if stage == 2:
                        wsum(j - 1)
                rowmaxes[j] = mask_rowmax(j)
            tail_a(SLOTS - 1)
            tail_b(SLOTS - 1)
            wsum(SLOTS - 1)
            nc.sync.dma_start(out=outp[:, :], in_=out_sb)
    nc.finalize()
    return nc


def kernel(seq, lengths, key_w, query_w, bias):
    global LAST_EXEC_NS
    seq = np.asarray(seq, dtype=np.float32)
    lengths_np = np.asarray(lengths).astype(np.int64)
    key_w = np.asarray(key_w, dtype=np.float32)
    query_w = np.asarray(query_w, dtype=np.float32)
    bias = np.asarray(bias, dtype=np.float32)

    order = np.argsort(-lengths_np, kind="stable")  # descending length
    n_desc = []
    for j in range(SLOTS):
        grp = order[j * NCORES : (j + 1) * NCORES]
        n_desc.append(max(1, int(-(-int(lengths_np[grp].max()) // 128))))
    # Small slots first (cheap prologue, PE transposes while the loads
    # stream), then the big slots descending (xbar transposes on the
    # now-idle DMA engines), with the smallest slot last so the exposed
    # final softmax/weighted-sum tail is tiny.
    asc = sorted(range(SLOTS), key=lambda j: n_desc[j])
    slot_perm = (
        asc[1 : SLOTS - 3]
        + [asc[SLOTS - 1], asc[SLOTS - 2], asc[SLOTS - 3], asc[0]]
    )
    n_list = [n_desc[j] for j in slot_perm]
    key = tuple(n_list)
    if key not in _PROG_CACHE:
        _PROG_CACHE[key] = _build_program(n_list)
    nc = _PROG_CACHE[key]

    amat_np = (key_w @ query_w.T + query_w @ key_w.T) * 0.5  # [D, D] symmetric
    # logits = |x(K+Q)/2|^2 - |x(K-Q)/2|^2 exactly equals (xK).(xQ)
    kqpm_np = np.ascontiguousarray(
        np.concatenate([(key_w + query_w) * 0.5, (key_w - query_w) * 0.5], axis=1)
    )
    in_maps = []
    for i in range(NCORES):
        m = {"amat": amat_np, "kqpm": kqpm_np}
        mask_cols = []
        for js, jd in enumerate(slot_perm):
            n = n_list[js]
            b = int(order[jd * NCORES + i])
            m[f"x{js}"] = seq[b, : n * 128, :].reshape(n, 128, D)
            lb = int(lengths_np[b])
            col = np.where(np.arange(n * 128) < lb, 0.0, -1e30).astype(np.float32)
            mask_cols.append(col.reshape(n, 128).T)  # [128, n]
        m["mask"] = np.ascontiguousarray(np.concatenate(mask_cols, axis=1))
        in_maps.append(m)

    trace = os.environ.get("KQA_TRACE") == "1"
    res = run_bass_kernel_spmd(
        nc, in_maps, core_ids=list(range(NCORES)), trace=trace
    )
    LAST_EXEC_NS = res.exec_time_ns

    out = np.empty((B, D), dtype=np.float32)
    for i in range(NCORES):
        r = res.results[i]["out"]  # [128, 2*SLOTS]
        for js, jd in enumerate(slot_perm):
            b = int(order[jd * NCORES + i])
            acc = r[:, js]
            z = r[:, SLOTS + js].sum(dtype=np.float64)
            out[b] = (acc / z).astype(np.float32) + bias
    return out
